# revision 27
# baseline (speedup 1.0000x reference)
"""AttentiveMLP GNN message-passing kernel for 8 Trainium2 NeuronCores.

Sharding: edges are partitioned BY DESTINATION NODE (each core owns ~N/8 nodes
plus all their incoming edges) so no cross-core collectives are needed. Nodes
are grouped on the host into degree-bucket classes (a pure layout/permutation
choice); within a class every node has exactly d edge slots (pad slots carry
logit -60 -> weight ~0), so segment softmax and the attention-weighted
aggregation are static dense ops over [128, T*d] tiles.

Key device-side structure (chosen off the TRN2 DVE cost model):
 - softmax max-subtraction is dropped (logits ~N(0,1): exp() cannot overflow,
   result is mathematically identical), killing two full edge passes.
 - per-edge weighting is one bf16 tensor_tensor multiply (2x DVE mode).
 - the segment reduction over d runs as a pairwise halving tree of bf16
   tensor_tensor adds (2x mode) instead of tensor_reduce (which has no fast
   mode); d is decomposed into power-of-2 slot blocks (e.g. 22 = 16+4+2) so
   every level halves evenly, with small cross-block merge adds on GpSimd.
 - 1/denominator is folded into the single [nodes,16] normalization multiply.
 - ELU is computed as elu(x)+1 = min(exp(x),1) + relu(x) (two activations +
   one fused scalar_tensor_tensor); the +1 shift is folded into b1 on host.
 - logits ride bf16; edge/node features ride bf16 (fp8 node features halve
   DMA but cost 2.7% rel err - over the 2e-2 budget; fp8 edge features drop
   the DVE multiply out of its 2x mode, costing more than the DMA saved).
 - scheduling: all exps are queued on the Act engine upfront; the class loop
   is software-pipelined (stage1 = den/recip/prod/tree of class i+1 issues
   before stage2 = norm/transpose of class i) so the in-order Vector queue
   never stalls on a GpSimd tree; each ready block's ELU head (ctx matmul +
   s1/s2/cb) issues one block ahead of its MLP chunk loop; the last blocks
   alternate h/ov between Act and Vector.
"""
import os
import numpy as np
import ml_dtypes
from contextlib import ExitStack

import concourse.bass as bass
import concourse.bacc as bacc
import concourse.tile as tile
import concourse.mybir as mybir
from concourse.bass_utils import run_bass_kernel_spmd

N_NODES = 100000
N_EDGES = 1600000
EF = 16
HID = 32
NF = 128
NCORES = 8
CHUNK = 512

f32 = mybir.dt.float32
bf16 = mybir.dt.bfloat16
fp8 = mybir.dt.float8e4
BF = ml_dtypes.bfloat16
F8 = ml_dtypes.float8_e4m3fn

PAD_LG = -60.0  # exp(-60) ~ 8.8e-27: pad slots contribute ~0 weight, no inf/nan

BUCKETS = [8, 12, 14, 16, 18, 20, 24, 40,
           64, 128, 256, 1024, 4096, 65536, 1048576, 2097152]


def _bin_blocks(d):
    """Contiguous slot ranges of power-of-2 width covering [0, d), descending."""
    out, s = [], 0
    for k in range(21, -1, -1):
        w = 1 << k
        if d & w:
            out.append((s, w))
            s += w
    return out


def _bucket_of(deg):
    b = np.zeros_like(deg)
    nz = deg > 0
    idx = np.searchsorted(np.asarray(BUCKETS), deg[nz])
    b[nz] = np.asarray(BUCKETS)[idx]
    return b


def _build_plan(dst):
    deg = np.bincount(dst, minlength=N_NODES)
    deg = _bucket_of(deg)
    order = np.argsort(deg, kind="stable")
    sdeg = deg[order]
    uniq, starts, counts = np.unique(sdeg, return_index=True, return_counts=True)
    ncls = len(uniq)
    rank = np.arange(N_NODES) - np.repeat(starts, counts)
    dev = rank % NCORES
    row_in_class = rank // NCORES
    n_pad = (counts + NCORES - 1) // NCORES
    n_pad = ((n_pad + 127) // 128) * 128   # 128-aligned class rows/offsets

    cls_ids = [ci for ci in range(ncls) if uniq[ci] > 0]
    cls_ids.sort(key=lambda ci: int(uniq[ci]) * int(n_pad[ci]))
    if uniq[0] == 0:
        cls_ids = cls_ids + [0]
    offs_arr = np.zeros(ncls, dtype=np.int64)
    acc = 0
    for ci in cls_ids:
        offs_arr[ci] = acc
        acc += n_pad[ci]
    R = int(acc)

    cls_of_pos = np.repeat(np.arange(ncls), counts)
    lrow = offs_arr[cls_of_pos] + row_in_class

    node_dev = np.empty(N_NODES, dtype=np.int64)
    node_lrow = np.empty(N_NODES, dtype=np.int64)
    node_dev[order] = dev
    node_lrow[order] = lrow

    classes = [(int(uniq[ci]), int(n_pad[ci]), int(offs_arr[ci])) for ci in cls_ids]
    deg0_rows = classes[-1][1] if classes and classes[-1][0] == 0 else 0
    kclasses = [c for c in classes if c[0] > 0]
    zero_tail_start = R - deg0_rows

    n_tiles = R // 128
    nfull = n_tiles // 16
    tail = n_tiles - nfull * 16
    # block b covers tiles [16b, 16b+ntb); chunk g of block b is 32*ntb wide
    blocks = [(16 * b, 16) for b in range(nfull)]
    if tail:
        blocks.append((nfull * 16, tail))

    return dict(
        uniq=uniq, counts=counts, node_dev=node_dev, node_lrow=node_lrow,
        R=R, n_tiles=n_tiles, blocks=blocks, kclasses=kclasses,
        zero_tail_start=zero_tail_start,
    )


def _mlpcol(r, n_tiles):
    """node row -> column in the transposed-MLP [NF, R] layout."""
    r = np.asarray(r)
    t = r // 128
    b = t // 16
    lt = t % 16
    q = (r % 128) // 32
    c = r % 32
    nfull = n_tiles // 16
    tail = n_tiles - nfull * 16
    full = b < nfull
    col_full = (4 * b + q) * CHUNK + 32 * lt + c
    col_tail = nfull * 4 * CHUNK + q * (32 * tail) + 32 * lt + c
    return np.where(full, col_full, col_tail)


def _shard_inputs(inputs, plan):
    lg = np.ascontiguousarray(
        np.asarray(inputs["edge_logits"], dtype=np.float32).reshape(-1))
    ef = np.ascontiguousarray(np.asarray(inputs["edge_feats"], dtype=np.float32))
    nf = np.asarray(inputs["node_feats"], dtype=np.float32)
    dst = np.asarray(inputs["dst"])
    W_et = np.asarray(inputs["W_et"], dtype=np.float32)
    b_et = np.asarray(inputs["b_et"], dtype=np.float32)
    W1 = np.asarray(inputs["W1"], dtype=np.float32)
    b1 = np.asarray(inputs["b1"], dtype=np.float32)
    W2 = np.asarray(inputs["W2"], dtype=np.float32)
    b2 = np.asarray(inputs["b2"], dtype=np.float32)

    node_dev, node_lrow = plan["node_dev"], plan["node_lrow"]
    R, n_tiles = plan["R"], plan["n_tiles"]
    kclasses = plan["kclasses"]

    ekey = node_dev[dst] * R + node_lrow[dst]
    eorder = np.argsort(ekey, kind="stable")
    sk = ekey[eorder]
    newrun = np.empty(N_EDGES, dtype=bool)
    newrun[0] = True
    newrun[1:] = sk[1:] != sk[:-1]
    runstart = np.maximum.accumulate(np.where(newrun, np.arange(N_EDGES), 0))
    slot = np.arange(N_EDGES) - runstart
    e_dev = node_dev[dst[eorder]]
    e_lrow = node_lrow[dst[eorder]]
    lg_s = lg[eorder].astype(BF)
    ef_s = ef[eorder].astype(BF)

    in_maps = [dict() for _ in range(NCORES)]
    for dv in range(NCORES):
        dmask = e_dev == dv
        d_lrow = e_lrow[dmask]
        d_slot = slot[dmask]
        d_lg = lg_s[dmask]
        d_ef = ef_s[dmask]
        for idx, (d, npad, off) in enumerate(kclasses):
            T = (npad + 127) // 128
            cmask = (d_lrow >= off) & (d_lrow < off + npad)
            r = d_lrow[cmask] - off
            s = d_slot[cmask]
            p = r % 128
            t = r // 128
            # host layout [128, T*d]: row r=(t*128+p) -> [p, t*d + s]
            flat_lg = np.full((128, T * d), PAD_LG, dtype=BF)
            flat_lg[p, t * d + s] = d_lg[cmask]
            # feature-major slots: [p, t*d*EF + f*d + s]
            flat_ef = np.zeros((128, T * d * EF), dtype=BF)
            col = (t * d * EF + s)[:, None] + np.arange(EF)[None, :] * d
            flat_ef[p[:, None], col] = d_ef[cmask]
            in_maps[dv][f"lg{idx}"] = flat_lg
            in_maps[dv][f"ef{idx}"] = flat_ef

    for dv in range(NCORES):
        sel = node_dev == dv
        nid = np.nonzero(sel)[0]
        lr = node_lrow[sel]
        nf_dev = np.zeros((R, NF), dtype=np.float32)
        nf_dev[_mlpcol(lr, n_tiles)] = nf[nid]
        in_maps[dv]["nfT"] = np.ascontiguousarray(nf_dev.T).astype(BF)

    # device computes cb' = elu(ctx)+1 = min(exp(z),1)+relu(z); fold the -1
    # correction into b1: h = relu(W1c^T cb' + W1n^T nf + (b1 - colsum(W1c)))
    b1 = b1 - W1[:HID].sum(axis=0)
    wet4 = np.zeros((128, 128), dtype=BF)
    bet4 = np.zeros((128, 1), dtype=np.float32)
    for g in range(4):
        wet4[32 * g:32 * g + EF, 32 * g:32 * g + HID] = W_et.astype(BF)
        bet4[32 * g:32 * g + HID, 0] = b_et
    consts = {
        "wet4": wet4,
        "bet4": bet4,
        "w1c": np.ascontiguousarray(np.tile(W1[:HID], (4, 1))).astype(BF),
        "w1n": np.ascontiguousarray(W1[HID:]).astype(BF),
        "b1": b1.reshape(NF, 1).astype(np.float32),
        "w2": W2.astype(BF),
        "b2": b2.reshape(NF, 1).astype(np.float32),
    }
    for dv in range(NCORES):
        in_maps[dv].update({k: v.copy() for k, v in consts.items()})
    return in_maps


def _unshard(results, plan):
    node_dev, node_lrow = plan["node_dev"], plan["node_lrow"]
    n_tiles = plan["n_tiles"]
    out = np.empty((N_NODES, NF), dtype=np.float32)
    for dv in range(NCORES):
        sel = node_dev == dv
        nid = np.nonzero(sel)[0]
        lr = node_lrow[sel]
        out_dev = results[dv]["outT"].T.astype(np.float32)
        out[nid] = out_dev[_mlpcol(lr, n_tiles)]
    return out


def _build_kernel(plan):
    kclasses = plan["kclasses"]
    R = plan["R"]
    blocks = plan["blocks"]
    n_blocks = len(blocks)
    zts = plan["zero_tail_start"]

    nc = bacc.Bacc("TRN2", target_bir_lowering=False, debug=False,
                   num_devices=NCORES)

    lg_d, ef_d = [], []
    for idx, (d, npad, off) in enumerate(kclasses):
        T = (npad + 127) // 128
        lg_d.append(nc.dram_tensor(f"lg{idx}", [128, T * d], bf16, kind="ExternalInput"))
        ef_d.append(nc.dram_tensor(f"ef{idx}", [128, T * d * EF], bf16, kind="ExternalInput"))
    nfT_d = nc.dram_tensor("nfT", [NF, R], bf16, kind="ExternalInput")
    wet4_d = nc.dram_tensor("wet4", [128, 128], bf16, kind="ExternalInput")
    bet4_d = nc.dram_tensor("bet4", [128, 1], f32, kind="ExternalInput")
    w1c_d = nc.dram_tensor("w1c", [NF, NF], bf16, kind="ExternalInput")
    w1n_d = nc.dram_tensor("w1n", [NF, NF], bf16, kind="ExternalInput")
    b1_d = nc.dram_tensor("b1", [NF, 1], f32, kind="ExternalInput")
    w2_d = nc.dram_tensor("w2", [NF, NF], bf16, kind="ExternalInput")
    b2_d = nc.dram_tensor("b2", [NF, 1], f32, kind="ExternalInput")
    out_d = nc.dram_tensor("outT", [NF, R], bf16, kind="ExternalOutput")

    # column base of each block in the [NF, R] layout
    col_base = []
    acc = 0
    for (t0b, ntb) in blocks:
        col_base.append(acc)
        acc += 4 * 32 * ntb
    assert acc == R

    with tile.TileContext(nc) as tc, ExitStack() as ctx:
        const_pool = ctx.enter_context(tc.tile_pool(name="const", bufs=1))
        agg_pool = ctx.enter_context(tc.tile_pool(name="agg", bufs=1))
        cls_pool = ctx.enter_context(tc.tile_pool(name="cls", bufs=4))
        work_pool = ctx.enter_context(tc.tile_pool(name="work", bufs=3))
        tree_pool = ctx.enter_context(tc.tile_pool(name="tree", bufs=3))
        small_pool = ctx.enter_context(tc.tile_pool(name="small", bufs=4))
        mlp_pool = ctx.enter_context(tc.tile_pool(name="mlp", bufs=3))
        ctx_pool = ctx.enter_context(tc.tile_pool(name="ctxs", bufs=2))
        ctx_psum = ctx.enter_context(tc.tile_pool(name="ctxp", bufs=2, space="PSUM"))
        mlp1_psum = ctx.enter_context(tc.tile_pool(name="m1p", bufs=4, space="PSUM"))
        mlp2_psum = ctx.enter_context(tc.tile_pool(name="m2p", bufs=2, space="PSUM"))

        # All logits are tiny (~5KB/partition total): DMA them all upfront on
        # the Pool queue and queue every exp at the head of the Act engine's
        # in-order queue, so no class's chain ever stalls behind MLP work.
        x_tiles = []
        for ci, (dc, npadc, _) in enumerate(kclasses):
            Tc = (npadc + 127) // 128
            lgt = cls_pool.tile([128, Tc * dc], bf16, tag=f"lgx{ci}", name=f"lgt{ci}")
            nc.sync.dma_start(lgt[:], lg_d[ci].ap())
            x_tiles.append(lgt)
        for ci, (dc, npadc, _) in enumerate(kclasses):
            Tc = (npadc + 127) // 128
            xt = cls_pool.tile([128, Tc * dc], bf16, tag=f"x{ci}", name=f"x{ci}")
            nc.scalar.activation(xt[:], x_tiles[ci][:],
                                 mybir.ActivationFunctionType.Exp)
            x_tiles[ci] = xt
        prefetched = {}
        for pi in range(min(3, len(kclasses))):
            dp, npadp, _ = kclasses[pi]
            Tp = (npadp + 127) // 128
            eftp = cls_pool.tile([128, Tp * dp * EF], bf16, tag="ef", name=f"eft_pre{pi}")
            nc.sync.dma_start(eftp[:], ef_d[pi].ap())
            prefetched[pi] = eftp

        def load_const(name, dram, shape, dtype=f32):
            t = const_pool.tile(shape, dtype, name=name)
            nc.gpsimd.dma_start(t[:], dram.ap())
            return t

        wet4 = load_const("wet4c", wet4_d, [128, 128], bf16)
        bet4 = load_const("bet4c", bet4_d, [128, 1])
        w1c = load_const("w1cc", w1c_d, [NF, NF], bf16)
        w1n = load_const("w1nc", w1n_d, [NF, NF], bf16)
        w2 = load_const("w2c", w2_d, [NF, NF], bf16)
        b1 = load_const("b1c", b1_d, [NF, 1])
        b2 = load_const("b2c", b2_d, [NF, 1])
        zeros = const_pool.tile([128, CHUNK], f32, name="zeros")
        nc.gpsimd.memset(zeros[:], 0.0)

        aggT_blocks = []
        for b, (t0b, ntb) in enumerate(blocks):
            ab = agg_pool.tile([128, 32 * ntb], bf16, name=f"aggT{b}")
            aggT_blocks.append(ab)
        # deg0 tail rows (none for this input, but keep correct in general):
        # their aggT columns are never written by any class; zero them so the
        # ctx matmul sees context 0 (b_et==0 -> elu path produces cb'=1).
        if zts < R:
            t_lo = zts // 128
            for b, (t0b, ntb) in enumerate(blocks):
                lo = max(t0b, t_lo)
                hi = t0b + ntb
                if lo < hi:
                    nc.gpsimd.memset(
                        aggT_blocks[b][0:128, 32 * (lo - t0b):32 * (hi - t0b)], 0.0)

        # ---------------- Phase B+C: ELU + MLP per block ----------------
        nf_tiles = {}

        def prefetch_nf(b):
            if b >= n_blocks or b in nf_tiles:
                return
            t0b, ntb = blocks[b]
            W = 32 * ntb
            cb4 = col_base[b]
            t = mlp_pool.tile([NF, 4 * W], bf16, tag="nfblk", name=f"nfblk{b}")
            nc.sync.dma_start(t[:], nfT_d.ap()[:, cb4:cb4 + 4 * W])
            nf_tiles[b] = t

        cb_tiles = {}

        def emit_elu(b):
            t0b, ntb = blocks[b]
            W = 32 * ntb
            ctx4 = ctx_psum.tile([128, W], f32, tag="ctx4", name=f"ctx4_{b}")
            nc.tensor.matmul(ctx4[:], wet4[:], aggT_blocks[b][:])
            s1 = mlp_pool.tile([128, W], f32, tag="s1", name=f"s1_{b}")
            nc.scalar.activation(s1[:], ctx4[:], mybir.ActivationFunctionType.Relu,
                                 bias=bet4[:], scale=1.0)
            s2 = mlp_pool.tile([128, W], f32, tag="s2", name=f"s2_{b}")
            nc.scalar.activation(s2[:], ctx4[:], mybir.ActivationFunctionType.Exp,
                                 bias=bet4[:], scale=1.0)
            # cb' = elu(ctx)+1 = min(exp(z),1) + relu(z); the +1 shift is
            # compensated in b1 on the host
            cb = ctx_pool.tile([128, W], bf16, tag="cb", name=f"cb_{b}")
            nc.vector.scalar_tensor_tensor(cb[:], s2[:], 1.0, s1[:],
                                           mybir.AluOpType.min,
                                           mybir.AluOpType.add)
            cb_tiles[b] = cb
            prefetch_nf(b)
            prefetch_nf(b + 1)

        def emit_mlp(b):
            t0b, ntb = blocks[b]
            W = 32 * ntb
            cb4 = col_base[b]
            cb = cb_tiles.pop(b)
            nfblk = nf_tiles.pop(b)
            oblk = mlp_pool.tile([NF, 4 * W], bf16, tag="oblk", name=f"oblk{b}")
            for g in range(4):
                j = 4 * b + g
                nfb = nfblk[0:NF, g * W:(g + 1) * W]
                ps1 = mlp1_psum.tile([NF, W], f32, tag="ps1", name=f"ps1_{j}")
                if g < 3:
                    cbg = cb[32 * g:32 * (g + 1), :]
                    w1cg = w1c[32 * g:32 * (g + 1), :]
                else:
                    # PE stationary/moving base partition must be 0/32/64
                    cb3 = mlp_pool.tile([HID, W], bf16, tag="cb3", name=f"cb3_{j}")
                    nc.vector.tensor_copy(cb3[:], cb[96:128, :])
                    cbg = cb3[:]
                    w1cg = w1c[0:HID, :]
                nc.tensor.matmul(ps1[:], w1n[:], nfb, start=True, stop=False)
                nc.tensor.matmul(ps1[:], w1cg, cbg, start=False, stop=True)
                h = mlp_pool.tile([NF, W], bf16, tag="h", name=f"h{j}")
                if b >= n_blocks - 3 and j % 2 == 1:
                    nc.vector.scalar_tensor_tensor(h[:], ps1[:], b1[:],
                                                   zeros[0:128, 0:W],
                                                   mybir.AluOpType.add,
                                                   mybir.AluOpType.max)
                else:
                    nc.scalar.activation(h[:], ps1[:],
                                         mybir.ActivationFunctionType.Relu,
                                         bias=b1[:], scale=1.0)
                ps2 = mlp2_psum.tile([NF, W], f32, tag="ps2", name=f"ps2_{j}")
                nc.tensor.matmul(ps2[:], w2[:], h[:])
                ov = oblk[0:NF, g * W:(g + 1) * W]
                if b >= n_blocks - 3 and j % 2 == 0:
                    nc.vector.scalar_tensor_tensor(ov, ps2[:], b2[:],
                                                   zeros[0:128, 0:W],
                                                   mybir.AluOpType.add,
                                                   mybir.AluOpType.max)
                else:
                    nc.scalar.activation(ov, ps2[:],
                                         mybir.ActivationFunctionType.Relu,
                                         bias=b2[:], scale=1.0)
            nc.gpsimd.dma_start(out_d.ap()[:, cb4:cb4 + 4 * W], oblk[:])

        next_block = [0]

        # ------------- Phase A: per-class segment softmax + aggregation ------
        # Two-stage software pipeline: stage1 (den/recip/prod/tree) of class
        # i+1 is ISSUED before stage2 (norm/transpose/emit) of class i, so the
        # in-order Vector queue always holds independent work ahead of any op
        # that waits on a GpSimd tree.
        stage_state = {}

        def stage1(idx):
            d, npad, off = kclasses[idx]
            T = (npad + 127) // 128
            if idx in prefetched:
                eft = prefetched[idx]
            else:
                eft = cls_pool.tile([128, T * d * EF], bf16, tag="ef", name=f"eft{idx}")
                nc.sync.dma_start(eft[:], ef_d[idx].ap())

            dblocks = _bin_blocks(d)
            x3 = x_tiles[idx][:].rearrange("p (t d) -> p t d", t=T)

            # den[p,t] = sum_d x  (single 1x reduce: tiny, fewer instructions)
            den = small_pool.tile([128, T], f32, tag="den", name=f"den{idx}")
            nc.vector.tensor_reduce(den[:], x3, mybir.AxisListType.X,
                                    mybir.AluOpType.add)
            rd = small_pool.tile([128, T], f32, tag="rd", name=f"rd{idx}")
            nc.vector.reciprocal(rd[:], den[:])

            # prod[p,t,f,s] = ef * x  (unnormalized; bf16 2x mode)
            prod = work_pool.tile([128, T * EF * d], bf16, tag="prod", name=f"prod{idx}")
            ef4 = eft[:].rearrange("p (t f d) -> p t f d", t=T, f=EF)
            x4b = x3.unsqueeze(2).broadcast_to([128, T, EF, d])
            prod4 = prod[:].rearrange("p (t f d) -> p t f d", t=T, f=EF)
            nc.vector.tensor_tensor(prod4, ef4, x4b, mybir.AluOpType.mult)

            # segment sum over d. Big classes: pairwise-halving tree of bf16
            # tensor_tensor adds (2x DVE mode; tensor_reduce has none). The
            # two mid-size classes ride GpSimd to offload the Vector engine;
            # small classes use one 1x tensor_reduce (instruction count wins).
            use_tree = T * EF * d >= 3000
            tree_eng = nc.gpsimd if d == 12 else nc.vector
            if use_tree:
                agg_parts = []
                for bi, (s0, w) in enumerate(dblocks):
                    cur = prod4[:, :, :, s0:s0 + w]
                    cw = w
                    while cw > 1:
                        half = cw // 2
                        nt = tree_pool.tile([128, T * EF * half], bf16,
                                            tag=f"ag{bi}_{half}", name=f"ag{idx}_{bi}_{half}")
                        nt4 = nt[:].rearrange("p (t f w) -> p t f w", t=T, f=EF)
                        lvl_eng = tree_eng
                        if d == 16 and half <= 4:
                            lvl_eng = nc.gpsimd
                        lvl_eng.tensor_tensor(nt4, cur[:, :, :, 0:half],
                                              cur[:, :, :, half:cw],
                                              mybir.AluOpType.add)
                        cur = nt4
                        cw = half
                    agg_parts.append(cur)
                a16u = agg_parts[0]
                for k in range(1, len(agg_parts)):
                    ns = tree_pool.tile([128, T * EF], bf16, tag=f"am{k}", name=f"am{idx}_{k}")
                    ns4 = ns[:].rearrange("p (t f w) -> p t f w", t=T, f=EF)
                    tree_eng.tensor_tensor(ns4, a16u, agg_parts[k],
                                           mybir.AluOpType.add)
                    a16u = ns4
                a16u2 = a16u.rearrange("p t f w -> p t (f w)")
            else:
                a16r = tree_pool.tile([128, T * EF], f32, tag="a16r", name=f"a16r{idx}")
                nc.vector.tensor_reduce(a16r[:].rearrange("p (t f) -> p t f", t=T),
                                        prod4, mybir.AxisListType.X,
                                        mybir.AluOpType.add)
                a16u2 = a16r[:].rearrange("p (t f) -> p t f", t=T)
            stage_state[idx] = (a16u2, rd, T, off)

        def stage2(idx):
            a16u2, rd, T, off = stage_state.pop(idx)
            # normalize: a16n[p,t,f] = a16u * (1/den), bf16
            a16n = work_pool.tile([128, T * EF], bf16, tag="a16n", name=f"a16n{idx}")
            a16n3 = a16n[:].rearrange("p (t f) -> p t f", t=T)
            rd_b = rd[:].unsqueeze(2).broadcast_to([128, T, EF])
            nc.vector.tensor_tensor(a16n3, a16u2, rd_b, mybir.AluOpType.mult)

            # 32x32 stream transpose into the aggT blocks. The 16 real features
            # are duplicated (stride-0 dim) to fill each 32-wide block; the
            # duplicate lands in aggT rows whose wet4 weights are zero.
            t0 = off // 128
            t = 0
            while t < T:
                b = (t0 + t) // 16
                t0b, ntb = blocks[b]
                te = min(T, t0b + ntb - t0)
                span = te - t
                in_ap = bass.AP(a16n.tensor, t * EF,
                                [[T * EF, 128], [EF, span], [0, 2], [1, EF]])
                nc.vector.transpose(
                    aggT_blocks[b][0:128, 32 * (t0 + t - t0b):32 * (t0 + te - t0b)],
                    in_ap)
                t = te
                # a block whose last tile was just transposed can start its
                # ELU+MLP now; this class's later transposes aren't its deps
                done_rows = 128 * (t0 + t)
                if idx == len(kclasses) - 1 and t == T:
                    done_rows = R * 2
                while (next_block[0] < n_blocks
                       and done_rows >= 128 * (blocks[next_block[0]][0]
                                               + blocks[next_block[0]][1])):
                    nb = next_block[0]
                    emit_elu(nb)
                    if nb > 0:
                        emit_mlp(nb - 1)
                    next_block[0] += 1

        for idx in range(len(kclasses)):
            stage1(idx)
            if idx > 0:
                stage2(idx - 1)
        stage2(len(kclasses) - 1)
        emit_mlp(n_blocks - 1)

    nc.compile()
    return nc


def kernel(**inputs):
    dst = np.asarray(inputs["dst"])
    plan = _build_plan(dst)
    in_maps = _shard_inputs(inputs, plan)
    nc = _build_kernel(plan)
    trace = bool(int(os.environ.get("GNN_PROFILE", "0")))
    if trace:
        try:
            _install_ntff_hook()
        except Exception:
            pass
    res = run_bass_kernel_spmd(nc, in_maps, core_ids=list(range(NCORES)),
                               trace=trace)
    kernel.last_results = res
    return _unshard(res.results, plan)


def _install_ntff_hook():
    """Recreate antenv.axon_hooks (absent in this image) so
    run_bass_kernel_spmd(trace=True) can NTFF-profile via libaxon_pjrt.so."""
    import contextlib, ctypes, sys, types
    if 'antenv.axon_hooks' in sys.modules:
        return
    lib = ctypes.CDLL('/opt/axon/libaxon_pjrt.so')
    lib.axon_start_nrt_profile.argtypes = [ctypes.POINTER(ctypes.c_int64), ctypes.c_size_t]
    lib.axon_start_nrt_profile.restype = ctypes.c_int64
    lib.axon_stop_nrt_profile.argtypes = [ctypes.c_char_p]
    lib.axon_stop_nrt_profile.restype = ctypes.c_int64

    @contextlib.contextmanager
    def _hook(output_dir, device_ids):
        import jax
        jax.devices()
        if device_ids:
            ids = (ctypes.c_int64 * len(device_ids))(*device_ids)
            rc = lib.axon_start_nrt_profile(ids, len(device_ids))
        else:
            rc = lib.axon_start_nrt_profile(None, 0)
        if rc != 0:
            raise RuntimeError(f"axon_start_nrt_profile rc={rc}")
        try:
            yield
        finally:
            n = lib.axon_stop_nrt_profile(str(output_dir).encode())
            print(f"ntff profile: {n} file(s) written to {output_dir}", file=sys.stderr)

    mod = types.ModuleType('antenv.axon_hooks')
    mod.get_axon_ntff_profile_hook = lambda: _hook
    mod.set_axon_ntff_profile_hook = lambda h: None
    import antenv
    antenv.axon_hooks = mod
    sys.modules['antenv.axon_hooks'] = mod


# revision 29
# speedup vs baseline: 1.0847x; 1.0847x over previous
"""AttentiveMLP GNN message-passing kernel for 8 Trainium2 NeuronCores.

Sharding: edges are partitioned BY DESTINATION NODE (each core owns ~N/8 nodes
plus all their incoming edges) so no cross-core collectives are needed. Nodes
are grouped on the host into degree-bucket classes (a pure layout/permutation
choice); within a class every node has exactly d edge slots (pad slots carry
logit -60 -> weight ~0), so segment softmax and the attention-weighted
aggregation are static dense ops over [128, T*d] tiles.

Key device-side structure (chosen off the TRN2 DVE cost model):
 - softmax max-subtraction is dropped (logits ~N(0,1): exp() cannot overflow,
   result is mathematically identical), killing two full edge passes.
 - per-edge weighting is one bf16 tensor_tensor multiply (2x DVE mode).
 - the segment reduction over d runs as a pairwise halving tree of bf16
   tensor_tensor adds (2x mode) instead of tensor_reduce (which has no fast
   mode); d is decomposed into power-of-2 slot blocks (e.g. 22 = 16+4+2) so
   every level halves evenly, with small cross-block merge adds on GpSimd.
 - 1/denominator is folded into the single [nodes,16] normalization multiply.
 - ELU is computed as elu(x)+1 = min(exp(x),1) + relu(x) (two activations +
   one fused scalar_tensor_tensor); the +1 shift is folded into b1 on host.
 - logits ride bf16; edge/node features ride bf16 (fp8 node features halve
   DMA but cost 2.7% rel err - over the 2e-2 budget; fp8 edge features drop
   the DVE multiply out of its 2x mode, costing more than the DMA saved).
 - scheduling: all exps are queued on the Act engine upfront; the class loop
   is software-pipelined (stage1 = den/recip/prod/tree of class i+1 issues
   before stage2 = norm/transpose of class i) so the in-order Vector queue
   never stalls on a GpSimd tree; each ready block's ELU head (ctx matmul +
   s1/s2/cb) issues one block ahead of its MLP chunk loop; the last blocks
   alternate h/ov between Act and Vector.
"""
import os
import numpy as np
import ml_dtypes
from contextlib import ExitStack

import concourse.bass as bass
import concourse.bacc as bacc
import concourse.tile as tile
import concourse.mybir as mybir
from concourse.bass_utils import run_bass_kernel_spmd

N_NODES = 100000
N_EDGES = 1600000
EF = 16
HID = 32
NF = 128
NCORES = 8
CHUNK = 512

f32 = mybir.dt.float32
bf16 = mybir.dt.bfloat16
fp8 = mybir.dt.float8e4
BF = ml_dtypes.bfloat16
F8 = ml_dtypes.float8_e4m3fn

PAD_LG = -60.0  # exp(-60) ~ 8.8e-27: pad slots contribute ~0 weight, no inf/nan

BUCKETS = [8, 12, 14, 16, 18, 20, 24, 40,
           64, 128, 256, 1024, 4096, 65536, 1048576, 2097152]


def _bin_blocks(d):
    """Contiguous slot ranges of power-of-2 width covering [0, d), descending."""
    out, s = [], 0
    for k in range(21, -1, -1):
        w = 1 << k
        if d & w:
            out.append((s, w))
            s += w
    return out


def _bucket_of(deg):
    b = np.zeros_like(deg)
    nz = deg > 0
    idx = np.searchsorted(np.asarray(BUCKETS), deg[nz])
    b[nz] = np.asarray(BUCKETS)[idx]
    return b


def _build_plan(dst):
    deg = np.bincount(dst, minlength=N_NODES)
    deg = _bucket_of(deg)
    order = np.argsort(deg, kind="stable")
    sdeg = deg[order]
    uniq, starts, counts = np.unique(sdeg, return_index=True, return_counts=True)
    ncls = len(uniq)
    rank = np.arange(N_NODES) - np.repeat(starts, counts)
    dev = rank % NCORES
    row_in_class = rank // NCORES
    n_pad = (counts + NCORES - 1) // NCORES
    n_pad = ((n_pad + 127) // 128) * 128   # 128-aligned class rows/offsets

    cls_ids = [ci for ci in range(ncls) if uniq[ci] > 0]
    cls_ids.sort(key=lambda ci: int(uniq[ci]) * int(n_pad[ci]))
    if uniq[0] == 0:
        cls_ids = cls_ids + [0]
    offs_arr = np.zeros(ncls, dtype=np.int64)
    acc = 0
    for ci in cls_ids:
        offs_arr[ci] = acc
        acc += n_pad[ci]
    R = int(acc)

    cls_of_pos = np.repeat(np.arange(ncls), counts)
    lrow = offs_arr[cls_of_pos] + row_in_class

    node_dev = np.empty(N_NODES, dtype=np.int64)
    node_lrow = np.empty(N_NODES, dtype=np.int64)
    node_dev[order] = dev
    node_lrow[order] = lrow

    classes = [(int(uniq[ci]), int(n_pad[ci]), int(offs_arr[ci])) for ci in cls_ids]
    deg0_rows = classes[-1][1] if classes and classes[-1][0] == 0 else 0
    kclasses = [c for c in classes if c[0] > 0]
    zero_tail_start = R - deg0_rows

    n_tiles = R // 128
    nfull = n_tiles // 16
    tail = n_tiles - nfull * 16
    # block b covers tiles [16b, 16b+ntb); chunk g of block b is 32*ntb wide
    blocks = [(16 * b, 16) for b in range(nfull)]
    if tail:
        blocks.append((nfull * 16, tail))

    return dict(
        uniq=uniq, counts=counts, node_dev=node_dev, node_lrow=node_lrow,
        R=R, n_tiles=n_tiles, blocks=blocks, kclasses=kclasses,
        zero_tail_start=zero_tail_start,
    )


def _mlpcol(r, n_tiles):
    """node row -> column in the transposed-MLP [NF, R] layout."""
    r = np.asarray(r)
    t = r // 128
    b = t // 16
    lt = t % 16
    q = (r % 128) // 32
    c = r % 32
    nfull = n_tiles // 16
    tail = n_tiles - nfull * 16
    full = b < nfull
    col_full = (4 * b + q) * CHUNK + 32 * lt + c
    col_tail = nfull * 4 * CHUNK + q * (32 * tail) + 32 * lt + c
    return np.where(full, col_full, col_tail)


def _shard_inputs(inputs, plan):
    lg = np.ascontiguousarray(
        np.asarray(inputs["edge_logits"], dtype=np.float32).reshape(-1))
    ef = np.ascontiguousarray(np.asarray(inputs["edge_feats"], dtype=np.float32))
    nf = np.asarray(inputs["node_feats"], dtype=np.float32)
    dst = np.asarray(inputs["dst"])
    W_et = np.asarray(inputs["W_et"], dtype=np.float32)
    b_et = np.asarray(inputs["b_et"], dtype=np.float32)
    W1 = np.asarray(inputs["W1"], dtype=np.float32)
    b1 = np.asarray(inputs["b1"], dtype=np.float32)
    W2 = np.asarray(inputs["W2"], dtype=np.float32)
    b2 = np.asarray(inputs["b2"], dtype=np.float32)

    node_dev, node_lrow = plan["node_dev"], plan["node_lrow"]
    R, n_tiles = plan["R"], plan["n_tiles"]
    kclasses = plan["kclasses"]

    ekey = node_dev[dst] * R + node_lrow[dst]
    eorder = np.argsort(ekey, kind="stable")
    sk = ekey[eorder]
    newrun = np.empty(N_EDGES, dtype=bool)
    newrun[0] = True
    newrun[1:] = sk[1:] != sk[:-1]
    runstart = np.maximum.accumulate(np.where(newrun, np.arange(N_EDGES), 0))
    slot = np.arange(N_EDGES) - runstart
    e_dev = node_dev[dst[eorder]]
    e_lrow = node_lrow[dst[eorder]]
    lg_s = lg[eorder].astype(BF)
    ef_s = ef[eorder].astype(BF)

    in_maps = [dict() for _ in range(NCORES)]
    for dv in range(NCORES):
        dmask = e_dev == dv
        d_lrow = e_lrow[dmask]
        d_slot = slot[dmask]
        d_lg = lg_s[dmask]
        d_ef = ef_s[dmask]
        for idx, (d, npad, off) in enumerate(kclasses):
            T = (npad + 127) // 128
            cmask = (d_lrow >= off) & (d_lrow < off + npad)
            r = d_lrow[cmask] - off
            s = d_slot[cmask]
            p = r % 128
            t = r // 128
            # host layout [128, T*d]: row r=(t*128+p) -> [p, t*d + s]
            flat_lg = np.full((128, T * d), PAD_LG, dtype=BF)
            flat_lg[p, t * d + s] = d_lg[cmask]
            # feature-major slots: [p, t*d*EF + f*d + s]
            flat_ef = np.zeros((128, T * d * EF), dtype=BF)
            col = (t * d * EF + s)[:, None] + np.arange(EF)[None, :] * d
            flat_ef[p[:, None], col] = d_ef[cmask]
            in_maps[dv][f"lg{idx}"] = flat_lg
            in_maps[dv][f"ef{idx}"] = flat_ef

    for dv in range(NCORES):
        sel = node_dev == dv
        nid = np.nonzero(sel)[0]
        lr = node_lrow[sel]
        nf_dev = np.zeros((R, NF), dtype=np.float32)
        nf_dev[_mlpcol(lr, n_tiles)] = nf[nid]
        in_maps[dv]["nfT"] = np.ascontiguousarray(nf_dev.T).astype(BF)

    # device computes cb' = elu(ctx)+1 = min(exp(z),1)+relu(z); fold the -1
    # correction into b1: h = relu(W1c^T cb' + W1n^T nf + (b1 - colsum(W1c)))
    b1 = b1 - W1[:HID].sum(axis=0)
    wet4 = np.zeros((128, 128), dtype=BF)
    bet4 = np.zeros((128, 1), dtype=np.float32)
    for g in range(4):
        wet4[32 * g:32 * g + EF, 32 * g:32 * g + HID] = W_et.astype(BF)
        bet4[32 * g:32 * g + HID, 0] = b_et
    consts = {
        "wet4": wet4,
        "bet4": bet4,
        "w1c": np.ascontiguousarray(np.tile(W1[:HID], (4, 1))).astype(BF),
        "w1n": np.ascontiguousarray(W1[HID:]).astype(BF),
        "b1": b1.reshape(NF, 1).astype(np.float32),
        "w2": W2.astype(BF),
        "b2": b2.reshape(NF, 1).astype(np.float32),
    }
    for dv in range(NCORES):
        in_maps[dv].update({k: v.copy() for k, v in consts.items()})
    return in_maps


def _unshard(results, plan):
    node_dev, node_lrow = plan["node_dev"], plan["node_lrow"]
    n_tiles = plan["n_tiles"]
    out = np.empty((N_NODES, NF), dtype=np.float32)
    for dv in range(NCORES):
        sel = node_dev == dv
        nid = np.nonzero(sel)[0]
        lr = node_lrow[sel]
        out_dev = results[dv]["outT"].T.astype(np.float32)
        out[nid] = out_dev[_mlpcol(lr, n_tiles)]
    return out


def _build_kernel(plan):
    kclasses = plan["kclasses"]
    R = plan["R"]
    blocks = plan["blocks"]
    n_blocks = len(blocks)
    zts = plan["zero_tail_start"]

    nc = bacc.Bacc("TRN2", target_bir_lowering=False, debug=False,
                   num_devices=NCORES)

    lg_d, ef_d = [], []
    for idx, (d, npad, off) in enumerate(kclasses):
        T = (npad + 127) // 128
        lg_d.append(nc.dram_tensor(f"lg{idx}", [128, T * d], bf16, kind="ExternalInput"))
        ef_d.append(nc.dram_tensor(f"ef{idx}", [128, T * d * EF], bf16, kind="ExternalInput"))
    nfT_d = nc.dram_tensor("nfT", [NF, R], bf16, kind="ExternalInput")
    wet4_d = nc.dram_tensor("wet4", [128, 128], bf16, kind="ExternalInput")
    bet4_d = nc.dram_tensor("bet4", [128, 1], f32, kind="ExternalInput")
    w1c_d = nc.dram_tensor("w1c", [NF, NF], bf16, kind="ExternalInput")
    w1n_d = nc.dram_tensor("w1n", [NF, NF], bf16, kind="ExternalInput")
    b1_d = nc.dram_tensor("b1", [NF, 1], f32, kind="ExternalInput")
    w2_d = nc.dram_tensor("w2", [NF, NF], bf16, kind="ExternalInput")
    b2_d = nc.dram_tensor("b2", [NF, 1], f32, kind="ExternalInput")
    out_d = nc.dram_tensor("outT", [NF, R], bf16, kind="ExternalOutput")

    # column base of each block in the [NF, R] layout
    col_base = []
    acc = 0
    for (t0b, ntb) in blocks:
        col_base.append(acc)
        acc += 4 * 32 * ntb
    assert acc == R

    with tile.TileContext(nc) as tc, ExitStack() as ctx:
        const_pool = ctx.enter_context(tc.tile_pool(name="const", bufs=1))
        agg_pool = ctx.enter_context(tc.tile_pool(name="agg", bufs=1))
        cls_pool = ctx.enter_context(tc.tile_pool(name="cls", bufs=4))
        work_pool = ctx.enter_context(tc.tile_pool(name="work", bufs=3))
        tree_pool = ctx.enter_context(tc.tile_pool(name="tree", bufs=3))
        small_pool = ctx.enter_context(tc.tile_pool(name="small", bufs=4))
        mlp_pool = ctx.enter_context(tc.tile_pool(name="mlp", bufs=3))
        ctx_pool = ctx.enter_context(tc.tile_pool(name="ctxs", bufs=2))
        ctx_psum = ctx.enter_context(tc.tile_pool(name="ctxp", bufs=2, space="PSUM"))
        mlp1_psum = ctx.enter_context(tc.tile_pool(name="m1p", bufs=4, space="PSUM"))
        mlp2_psum = ctx.enter_context(tc.tile_pool(name="m2p", bufs=2, space="PSUM"))

        # All logits are tiny (~5KB/partition total): DMA them all upfront on
        # the Pool queue and queue every exp at the head of the Act engine's
        # in-order queue, so no class's chain ever stalls behind MLP work.
        x_tiles = []
        for ci, (dc, npadc, _) in enumerate(kclasses):
            Tc = (npadc + 127) // 128
            lgt = cls_pool.tile([128, Tc * dc], bf16, tag=f"lgx{ci}", name=f"lgt{ci}")
            nc.gpsimd.dma_start(lgt[:], lg_d[ci].ap())
            x_tiles.append(lgt)
        for ci, (dc, npadc, _) in enumerate(kclasses):
            Tc = (npadc + 127) // 128
            xt = cls_pool.tile([128, Tc * dc], bf16, tag=f"x{ci}", name=f"x{ci}")
            nc.scalar.activation(xt[:], x_tiles[ci][:],
                                 mybir.ActivationFunctionType.Exp)
            x_tiles[ci] = xt
        prefetched = {}
        for pi in range(min(3, len(kclasses))):
            dp, npadp, _ = kclasses[pi]
            Tp = (npadp + 127) // 128
            eftp = cls_pool.tile([128, Tp * dp * EF], bf16, tag="ef", name=f"eft_pre{pi}")
            nc.sync.dma_start(eftp[:], ef_d[pi].ap())
            prefetched[pi] = eftp

        def load_const(name, dram, shape, dtype=f32):
            t = const_pool.tile(shape, dtype, name=name)
            nc.gpsimd.dma_start(t[:], dram.ap())
            return t

        wet4 = load_const("wet4c", wet4_d, [128, 128], bf16)
        bet4 = load_const("bet4c", bet4_d, [128, 1])
        w1c = load_const("w1cc", w1c_d, [NF, NF], bf16)
        w1n = load_const("w1nc", w1n_d, [NF, NF], bf16)
        w2 = load_const("w2c", w2_d, [NF, NF], bf16)
        b1 = load_const("b1c", b1_d, [NF, 1])
        b2 = load_const("b2c", b2_d, [NF, 1])
        zeros = const_pool.tile([128, CHUNK], f32, name="zeros")
        nc.gpsimd.memset(zeros[:], 0.0)

        aggT_blocks = []
        for b, (t0b, ntb) in enumerate(blocks):
            ab = agg_pool.tile([128, 32 * ntb], bf16, name=f"aggT{b}")
            aggT_blocks.append(ab)
        # deg0 tail rows (none for this input, but keep correct in general):
        # their aggT columns are never written by any class; zero them so the
        # ctx matmul sees context 0 (b_et==0 -> elu path produces cb'=1).
        if zts < R:
            t_lo = zts // 128
            for b, (t0b, ntb) in enumerate(blocks):
                lo = max(t0b, t_lo)
                hi = t0b + ntb
                if lo < hi:
                    nc.gpsimd.memset(
                        aggT_blocks[b][0:128, 32 * (lo - t0b):32 * (hi - t0b)], 0.0)

        # ---------------- Phase B+C: ELU + MLP per block ----------------
        nf_tiles = {}

        def prefetch_nf(b):
            if b >= n_blocks or b in nf_tiles:
                return
            t0b, ntb = blocks[b]
            W = 32 * ntb
            cb4 = col_base[b]
            t = mlp_pool.tile([NF, 4 * W], bf16, tag="nfblk", name=f"nfblk{b}")
            nc.sync.dma_start(t[:], nfT_d.ap()[:, cb4:cb4 + 4 * W])
            nf_tiles[b] = t

        cb_tiles = {}

        def emit_elu(b):
            t0b, ntb = blocks[b]
            W = 32 * ntb
            ctx4 = ctx_psum.tile([128, W], f32, tag="ctx4", name=f"ctx4_{b}")
            nc.tensor.matmul(ctx4[:], wet4[:], aggT_blocks[b][:])
            s1 = mlp_pool.tile([128, W], f32, tag="s1", name=f"s1_{b}")
            nc.scalar.activation(s1[:], ctx4[:], mybir.ActivationFunctionType.Relu,
                                 bias=bet4[:], scale=1.0)
            s2 = mlp_pool.tile([128, W], f32, tag="s2", name=f"s2_{b}")
            nc.scalar.activation(s2[:], ctx4[:], mybir.ActivationFunctionType.Exp,
                                 bias=bet4[:], scale=1.0)
            # cb' = elu(ctx)+1 = min(exp(z),1) + relu(z); the +1 shift is
            # compensated in b1 on the host
            cb = ctx_pool.tile([128, W], bf16, tag="cb", name=f"cb_{b}")
            nc.vector.scalar_tensor_tensor(cb[:], s2[:], 1.0, s1[:],
                                           mybir.AluOpType.min,
                                           mybir.AluOpType.add)
            cb_tiles[b] = cb
            prefetch_nf(b)
            prefetch_nf(b + 1)

        def emit_mlp(b):
            t0b, ntb = blocks[b]
            W = 32 * ntb
            cb4 = col_base[b]
            cb = cb_tiles.pop(b)
            nfblk = nf_tiles.pop(b)
            oblk = mlp_pool.tile([NF, 4 * W], bf16, tag="oblk", name=f"oblk{b}")
            for g in range(4):
                j = 4 * b + g
                nfb = nfblk[0:NF, g * W:(g + 1) * W]
                ps1 = mlp1_psum.tile([NF, W], f32, tag="ps1", name=f"ps1_{j}")
                if g < 3:
                    cbg = cb[32 * g:32 * (g + 1), :]
                    w1cg = w1c[32 * g:32 * (g + 1), :]
                else:
                    # PE stationary/moving base partition must be 0/32/64
                    cb3 = mlp_pool.tile([HID, W], bf16, tag="cb3", name=f"cb3_{j}")
                    nc.scalar.copy(cb3[:], cb[96:128, :])
                    cbg = cb3[:]
                    w1cg = w1c[0:HID, :]
                nc.tensor.matmul(ps1[:], w1n[:], nfb, start=True, stop=False)
                nc.tensor.matmul(ps1[:], w1cg, cbg, start=False, stop=True)
                h = mlp_pool.tile([NF, W], bf16, tag="h", name=f"h{j}")
                if b >= n_blocks - 3 and j % 2 == 1:
                    nc.vector.scalar_tensor_tensor(h[:], ps1[:], b1[:],
                                                   zeros[0:128, 0:W],
                                                   mybir.AluOpType.add,
                                                   mybir.AluOpType.max)
                else:
                    nc.scalar.activation(h[:], ps1[:],
                                         mybir.ActivationFunctionType.Relu,
                                         bias=b1[:], scale=1.0)
                ps2 = mlp2_psum.tile([NF, W], f32, tag="ps2", name=f"ps2_{j}")
                nc.tensor.matmul(ps2[:], w2[:], h[:])
                ov = oblk[0:NF, g * W:(g + 1) * W]
                if b >= n_blocks - 3 and j % 2 == 0:
                    nc.vector.scalar_tensor_tensor(ov, ps2[:], b2[:],
                                                   zeros[0:128, 0:W],
                                                   mybir.AluOpType.add,
                                                   mybir.AluOpType.max)
                else:
                    nc.scalar.activation(ov, ps2[:],
                                         mybir.ActivationFunctionType.Relu,
                                         bias=b2[:], scale=1.0)
            nc.gpsimd.dma_start(out_d.ap()[:, cb4:cb4 + 4 * W], oblk[:])

        next_block = [0]

        # ------------- Phase A: per-class segment softmax + aggregation ------
        # Two-stage software pipeline: stage1 (den/recip/prod/tree) of class
        # i+1 is ISSUED before stage2 (norm/transpose/emit) of class i, so the
        # in-order Vector queue always holds independent work ahead of any op
        # that waits on a GpSimd tree.
        stage_state = {}

        def stage1(idx):
            d, npad, off = kclasses[idx]
            T = (npad + 127) // 128
            if idx in prefetched:
                eft = prefetched[idx]
            else:
                eft = cls_pool.tile([128, T * d * EF], bf16, tag="ef", name=f"eft{idx}")
                nc.sync.dma_start(eft[:], ef_d[idx].ap())

            dblocks = _bin_blocks(d)
            x3 = x_tiles[idx][:].rearrange("p (t d) -> p t d", t=T)

            # den[p,t] = sum_d x  (single 1x reduce: tiny, fewer instructions)
            den = small_pool.tile([128, T], f32, tag="den", name=f"den{idx}")
            nc.vector.tensor_reduce(den[:], x3, mybir.AxisListType.X,
                                    mybir.AluOpType.add)
            rd = small_pool.tile([128, T], f32, tag="rd", name=f"rd{idx}")
            nc.vector.reciprocal(rd[:], den[:])

            # prod[p,t,f,s] = ef * x  (unnormalized; bf16 2x mode)
            prod = work_pool.tile([128, T * EF * d], bf16, tag="prod", name=f"prod{idx}")
            ef4 = eft[:].rearrange("p (t f d) -> p t f d", t=T, f=EF)
            x4b = x3.unsqueeze(2).broadcast_to([128, T, EF, d])
            prod4 = prod[:].rearrange("p (t f d) -> p t f d", t=T, f=EF)
            nc.vector.tensor_tensor(prod4, ef4, x4b, mybir.AluOpType.mult)

            # segment sum over d. Big classes: pairwise-halving tree of bf16
            # tensor_tensor adds (2x DVE mode; tensor_reduce has none). The
            # two mid-size classes ride GpSimd to offload the Vector engine;
            # small classes use one 1x tensor_reduce (instruction count wins).
            use_tree = T * EF * d >= 3000
            tree_eng = nc.gpsimd if d == 12 else nc.vector
            if use_tree:
                agg_parts = []
                for bi, (s0, w) in enumerate(dblocks):
                    cur = prod4[:, :, :, s0:s0 + w]
                    cw = w
                    while cw > 1:
                        half = cw // 2
                        nt = tree_pool.tile([128, T * EF * half], bf16,
                                            tag=f"ag{bi}_{half}", name=f"ag{idx}_{bi}_{half}")
                        nt4 = nt[:].rearrange("p (t f w) -> p t f w", t=T, f=EF)
                        tree_eng.tensor_tensor(nt4, cur[:, :, :, 0:half],
                                               cur[:, :, :, half:cw],
                                               mybir.AluOpType.add)
                        cur = nt4
                        cw = half
                    agg_parts.append(cur)
                a16u = agg_parts[0]
                for k in range(1, len(agg_parts)):
                    ns = tree_pool.tile([128, T * EF], bf16, tag=f"am{k}", name=f"am{idx}_{k}")
                    ns4 = ns[:].rearrange("p (t f w) -> p t f w", t=T, f=EF)
                    tree_eng.tensor_tensor(ns4, a16u, agg_parts[k],
                                           mybir.AluOpType.add)
                    a16u = ns4
                a16u2 = a16u.rearrange("p t f w -> p t (f w)")
            else:
                a16r = tree_pool.tile([128, T * EF], f32, tag="a16r", name=f"a16r{idx}")
                nc.vector.tensor_reduce(a16r[:].rearrange("p (t f) -> p t f", t=T),
                                        prod4, mybir.AxisListType.X,
                                        mybir.AluOpType.add)
                a16u2 = a16r[:].rearrange("p (t f) -> p t f", t=T)
            stage_state[idx] = (a16u2, rd, T, off)

        def stage2(idx):
            a16u2, rd, T, off = stage_state.pop(idx)
            # normalize: a16n[p,t,f] = a16u * (1/den), bf16
            a16n = work_pool.tile([128, T * EF], bf16, tag="a16n", name=f"a16n{idx}")
            a16n3 = a16n[:].rearrange("p (t f) -> p t f", t=T)
            rd_b = rd[:].unsqueeze(2).broadcast_to([128, T, EF])
            nc.vector.tensor_tensor(a16n3, a16u2, rd_b, mybir.AluOpType.mult)

            # 32x32 stream transpose into the aggT blocks. The 16 real features
            # are duplicated (stride-0 dim) to fill each 32-wide block; the
            # duplicate lands in aggT rows whose wet4 weights are zero.
            t0 = off // 128
            t = 0
            while t < T:
                b = (t0 + t) // 16
                t0b, ntb = blocks[b]
                te = min(T, t0b + ntb - t0)
                span = te - t
                in_ap = bass.AP(a16n.tensor, t * EF,
                                [[T * EF, 128], [EF, span], [0, 2], [1, EF]])
                nc.vector.transpose(
                    aggT_blocks[b][0:128, 32 * (t0 + t - t0b):32 * (t0 + te - t0b)],
                    in_ap)
                t = te
                # a block whose last tile was just transposed can start its
                # ELU+MLP now; this class's later transposes aren't its deps
                done_rows = 128 * (t0 + t)
                if idx == len(kclasses) - 1 and t == T:
                    done_rows = R * 2
                while (next_block[0] < n_blocks
                       and done_rows >= 128 * (blocks[next_block[0]][0]
                                               + blocks[next_block[0]][1])):
                    nb = next_block[0]
                    emit_elu(nb)
                    if nb > 0:
                        emit_mlp(nb - 1)
                    next_block[0] += 1

        for idx in range(len(kclasses)):
            stage1(idx)
            if idx > 0:
                stage2(idx - 1)
        stage2(len(kclasses) - 1)
        emit_mlp(n_blocks - 1)

    nc.compile()
    return nc


def kernel(**inputs):
    dst = np.asarray(inputs["dst"])
    plan = _build_plan(dst)
    in_maps = _shard_inputs(inputs, plan)
    nc = _build_kernel(plan)
    trace = bool(int(os.environ.get("GNN_PROFILE", "0")))
    if trace:
        try:
            _install_ntff_hook()
        except Exception:
            pass
    res = run_bass_kernel_spmd(nc, in_maps, core_ids=list(range(NCORES)),
                               trace=trace)
    kernel.last_results = res
    return _unshard(res.results, plan)


def _install_ntff_hook():
    """Recreate antenv.axon_hooks (absent in this image) so
    run_bass_kernel_spmd(trace=True) can NTFF-profile via libaxon_pjrt.so."""
    import contextlib, ctypes, sys, types
    if 'antenv.axon_hooks' in sys.modules:
        return
    lib = ctypes.CDLL('/opt/axon/libaxon_pjrt.so')
    lib.axon_start_nrt_profile.argtypes = [ctypes.POINTER(ctypes.c_int64), ctypes.c_size_t]
    lib.axon_start_nrt_profile.restype = ctypes.c_int64
    lib.axon_stop_nrt_profile.argtypes = [ctypes.c_char_p]
    lib.axon_stop_nrt_profile.restype = ctypes.c_int64

    @contextlib.contextmanager
    def _hook(output_dir, device_ids):
        import jax
        jax.devices()
        if device_ids:
            ids = (ctypes.c_int64 * len(device_ids))(*device_ids)
            rc = lib.axon_start_nrt_profile(ids, len(device_ids))
        else:
            rc = lib.axon_start_nrt_profile(None, 0)
        if rc != 0:
            raise RuntimeError(f"axon_start_nrt_profile rc={rc}")
        try:
            yield
        finally:
            n = lib.axon_stop_nrt_profile(str(output_dir).encode())
            print(f"ntff profile: {n} file(s) written to {output_dir}", file=sys.stderr)

    mod = types.ModuleType('antenv.axon_hooks')
    mod.get_axon_ntff_profile_hook = lambda: _hook
    mod.set_axon_ntff_profile_hook = lambda h: None
    import antenv
    antenv.axon_hooks = mod
    sys.modules['antenv.axon_hooks'] = mod
